# revision 7
# baseline (speedup 1.0000x reference)
"""Cross-attention kernel for 8 Trainium2 NeuronCores.

Problem: nn_CrossAttention (N=2, X=1024, T=4096, D=1024, H=16, hd=64).

Sharding: core c handles batch n = c//4 and head-group hg = c%4
(4 heads = 256 output dims). No cross-core communication.

Host prep per core (numpy, outside HW timing): transposed, per-partition
contiguous layouts so every DMA is 128 long linear descriptors.

Device (all matmuls contract over the partition dim):
  qT[c,x]  = wqT.T @ xT    (+bq)         kT[c,t] = wkT.T @ ctxT (+bk)
  v[t,c]   = ctxT.T @ wvT  (+bv via DVE broadcast add)
  S.T[t,x] = kT_h.T @ qT_h   (per head, K=64, head pairs packed into
                              array row-halves -> 2nd matmul rides free)
  P.T      = exp(S.T / 8)                 (ScalarE, scale folded in)
  O'.T[65,x] = [V_h | 1].T @ P.T          (ones col gives softmax denom)
  O[x,64]  = transpose(O'.T) rows 0:64 * 1/row64   (PE transpose + DVE)

Schedule: one software pipeline; the 128 exps (ScalarE, ~139us/core)
pace the attention steps while projection work rides as PE filler.
O' matmuls LAG one step behind scores/exp so they never stall on the
exp latency.  PSUM budget (8 banks): score tiles 2 slots x 2 banks,
O' accumulators 2 slots x 1, single-step projection bursts 2 slots x 1.
Input DMA issues are spread across four engine queues (descriptor
generation is ~0.6us per tensor and serializes per-queue).  Each
stream's drain (copy/transpose/scale + output DMA) is deferred into the
next stream's first steps; dummy matmuls warm the PE clock at start.
"""

import sys
import types

import numpy as np
import ml_dtypes
from contextlib import ExitStack

# If BASS_TRACE is set, concourse.bass_utils imports antenv.axon_hooks,
# which this image's antenv package lacks. Provide a no-op stub so
# tracing degrades gracefully instead of crashing (a real hook installed
# earlier by a test harness wins).
try:
    import antenv.axon_hooks  # noqa: F401
except ImportError:
    _m = types.ModuleType("antenv.axon_hooks")
    _m.get_axon_ntff_profile_hook = lambda: None
    _m.set_axon_ntff_profile_hook = lambda h: None
    sys.modules["antenv.axon_hooks"] = _m
    try:
        import antenv
        antenv.axon_hooks = _m
    except ImportError:
        pass

import concourse.bacc as bacc
import concourse.tile as tile
import concourse.mybir as mybir
from concourse.bass_utils import run_bass_kernel_spmd
from concourse.masks import make_identity

D, H, HD = 1024, 16, 64
N, X, T = 2, 1024, 4096
NCORES = 8
CH = 4            # heads per core
CW = CH * HD      # 256 output cols per core
KT = D // 128     # 8 d-tiles
TT = T // 128     # 32 t-tiles
XTILES = X // 128  # 8 x-tiles
BF16 = mybir.dt.bfloat16
F32 = mybir.dt.float32
EXP = mybir.ActivationFunctionType.Exp

_CACHE = {}


def _build_program():
    nc = bacc.Bacc("TRN2", target_bir_lowering=False, debug=False,
                   num_devices=NCORES)

    # per-partition contiguous layouts: each dma_start is 128 descriptors
    xt_d = nc.dram_tensor("xt", (128, 2, KT, 512), BF16, kind="ExternalInput")
    ctxt_d = nc.dram_tensor("ctxt", (128, 8, KT, 512), BF16,
                            kind="ExternalInput")
    wqt_d = nc.dram_tensor("wqt", (128, KT, CW), BF16, kind="ExternalInput")
    wkt_d = nc.dram_tensor("wkt", (128, KT, CW), BF16, kind="ExternalInput")
    wvt_d = nc.dram_tensor("wvt", (128, KT, CW), BF16, kind="ExternalInput")
    bq_d = nc.dram_tensor("bq", (128, 2), F32, kind="ExternalInput")
    bk_d = nc.dram_tensor("bk", (128, 2), F32, kind="ExternalInput")
    bv_d = nc.dram_tensor("bv", (128, CW), BF16, kind="ExternalInput")
    out_d = nc.dram_tensor("out", (128, XTILES, CW), F32,
                           kind="ExternalOutput")

    with tile.TileContext(nc) as tc, ExitStack() as ctx:
        consts = ctx.enter_context(tc.tile_pool(name="consts", bufs=1))
        pt_pool = ctx.enter_context(tc.tile_pool(name="pt", bufs=4))
        osb_pool = ctx.enter_context(tc.tile_pool(name="osb", bufs=2))
        rc_pool = ctx.enter_context(tc.tile_pool(name="rc", bufs=2))
        # PSUM budget: 8 banks total.
        # st: score tiles, 2 slots x 2 banks (double-buffered vs exp)
        st_pool = ctx.enter_context(
            tc.tile_pool(name="st", bufs=2, space="PSUM"))
        # oacc: O' accumulators, 2 slots x 1 bank
        oacc_pool = ctx.enter_context(
            tc.tile_pool(name="oacc", bufs=2, space="PSUM"))
        # rot: single-step bursts (proj chunks, v tiles, transposes), 2x1
        rot_pool = ctx.enter_context(
            tc.tile_pool(name="rot", bufs=2, space="PSUM"))

        # ---- resident SBUF tensors ----
        wq_sb = consts.tile([128, KT, CW], BF16)
        wk_sb = consts.tile([128, KT, CW], BF16)
        wv_sb = consts.tile([128, KT, CW], BF16)
        xt_sb = consts.tile([128, 2, KT, 512], BF16)
        ctx_sb = consts.tile([128, 8, KT, 512], BF16)
        qt_sb = consts.tile([128, 2, X], BF16)
        kt_sb = consts.tile([128, 2, T], BF16)
        vp_sb = consts.tile([128, TT, CH * (HD + 1)], BF16)  # [.., 260]
        out_sb = consts.tile([128, XTILES, CW], F32)
        bq_sb = consts.tile([128, 2], F32)
        bk_sb = consts.tile([128, 2], F32)
        bv_sb = consts.tile([128, CW], BF16)
        ident = consts.tile([128, 128], F32)
        dumin = consts.tile([128, 512], BF16)

        vp_h = vp_sb[:].rearrange("p t (h c) -> p t h c", c=HD + 1)
        bv_h = bv_sb[:].rearrange("p (h c) -> p h c", c=HD)

        # ---- input DMAs, spread across engine queues so descriptor
        # generation (~0.6us each) runs in parallel; highest-priority
        # tensors first within each queue ----
        nc.gpsimd.memset(dumin[:], 0.0)
        nc.sync.dma_start(wq_sb[:], wqt_d.ap())
        nc.sync.dma_start(xt_sb[:, 0], xt_d.ap()[:, 0])
        nc.sync.dma_start(bq_sb[:], bq_d.ap())
        nc.sync.dma_start(xt_sb[:, 1], xt_d.ap()[:, 1])
        nc.sync.dma_start(ctx_sb[:, 6], ctxt_d.ap()[:, 6])
        nc.sync.dma_start(ctx_sb[:, 7], ctxt_d.ap()[:, 7])
        nc.scalar.dma_start(wk_sb[:], wkt_d.ap())
        for c in (0, 2, 4):
            nc.scalar.dma_start(ctx_sb[:, c], ctxt_d.ap()[:, c])
        nc.gpsimd.dma_start(wv_sb[:], wvt_d.ap())
        for c in (1, 3, 5):
            nc.gpsimd.dma_start(ctx_sb[:, c], ctxt_d.ap()[:, c])
        nc.gpsimd.dma_start(bk_sb[:], bk_d.ap())
        nc.gpsimd.dma_start(bv_sb[:], bv_d.ap())

        # ---- PE warm-up: dummy matmuls while input DMAs land ----
        dps = rot_pool.tile([128, 512], F32, tag="rot", name="dps")
        for i in range(8):
            nc.tensor.matmul(dps[:], dumin[:, 0:128], dumin[:],
                             start=(i == 0), stop=(i == 7))
        make_identity(nc, ident[:])
        nc.gpsimd.memset(vp_h[:, :, :, HD:HD + 1], 1.0)

        # ---- projection bursts; each chunk is two 4-matmul halves so
        # exp-paced streams can take ~0.9us of filler per step ----
        def proj_halves(kind, ct, i):
            w_sb, dst_sb, b_sb = {
                "q": (wq_sb, qt_sb, bq_sb),
                "k": (wk_sb, kt_sb, bk_sb),
            }[kind]
            cell = {}

            def run(half):
                if half == 0:
                    cell["ps"] = rot_pool.tile(
                        [128, 512], F32, tag="rot", name=f"{kind}ps{ct}_{i}")
                ps = cell["ps"]
                for dt in range(4 * half, 4 * half + 4):
                    mov = (xt_sb[:, i, dt, :] if kind == "q"
                           else ctx_sb[:, i, dt, :])
                    nc.tensor.matmul(
                        ps[:], w_sb[:, dt, 128 * ct:128 * (ct + 1)], mov,
                        start=(dt == 0), stop=(dt == KT - 1))
                if half == 1:
                    nc.vector.tensor_scalar_add(
                        dst_sb[:, ct, 512 * i:512 * (i + 1)], ps[:],
                        b_sb[:, ct:ct + 1])

            return [lambda: run(0), lambda: run(1)]

        def qt_proj(ct, xc):
            for f in proj_halves("q", ct, xc):
                f()

        def kt_chunk(ct, c):
            for f in proj_halves("k", ct, c):
                f()

        def v_tile(tt):
            ps = rot_pool.tile([128, 512], F32, tag="rot", name=f"vps{tt}")
            c, sub = tt // 4, tt % 4
            for dt in range(KT):
                nc.tensor.matmul(
                    ps[:, 0:CW],
                    ctx_sb[:, c, dt, 128 * sub:128 * (sub + 1)],
                    wv_sb[:, dt, :],
                    start=(dt == 0), stop=(dt == KT - 1))
            nc.vector.tensor_add(
                vp_h[:, tt, :, 0:HD],
                ps[:, 0:CW].rearrange("p (h c) -> p h c", c=HD),
                bv_h[:])

        # ---- attention pieces ----
        oacc = {}   # (hp, xc) -> [tileA, tileB]
        pts = {}    # (hp, xc, tt) -> pt tile

        def scores(hp, xc, tt):
            st = st_pool.tile([128, 1024], F32, tag="st",
                              name=f"st{hp}{xc}{tt}")
            for h2 in range(2):
                nc.tensor.matmul(
                    st[:, 512 * h2:512 * (h2 + 1)],
                    kt_sb[64 * h2:64 * (h2 + 1), hp,
                          128 * tt:128 * (tt + 1)],
                    qt_sb[64 * h2:64 * (h2 + 1), hp,
                          512 * xc:512 * (xc + 1)],
                    start=True, stop=True)
            pt = pt_pool.tile([128, 1024], BF16, tag="pt",
                              name=f"pt{hp}{xc}{tt}")
            nc.scalar.activation(pt[:], st[:], EXP, scale=0.125)
            pts[(hp, xc, tt)] = pt

        def oprime(hp, xc, tt):
            if tt == 0:
                oacc[(hp, xc)] = [
                    oacc_pool.tile([65, 512], F32, tag="oacc",
                                   name=f"oacc{hp}{xc}{h2}")
                    for h2 in range(2)]
            pt = pts.pop((hp, xc, tt))
            for h2 in range(2):
                h = 2 * hp + h2
                nc.tensor.matmul(
                    oacc[(hp, xc)][h2][:],
                    vp_sb[:, tt, 65 * h:65 * (h + 1)],
                    pt[:, 512 * h2:512 * (h2 + 1)],
                    start=(tt == 0), stop=(tt == TT - 1))

        # ---- drain pieces (issued across the next stream's steps) ----
        def drain_copy(hp, xc):
            ots = []
            for h2 in range(2):
                ot = osb_pool.tile([65, 512], F32, tag="osb",
                                   name=f"ot{hp}{xc}{h2}")
                nc.vector.tensor_copy(ot[:], oacc[(hp, xc)][h2][:])
                ots.append(ot)
            del oacc[(hp, xc)]
            return ots

        def drain_sub(hp, xc, ots, s):
            for h2 in range(2):
                h = 2 * hp + h2
                tp = rot_pool.tile([128, 65], F32, tag="rot",
                                   name=f"tp{hp}{xc}{h2}{s}")
                nc.tensor.transpose(
                    tp[:], ots[h2][:, 128 * s:128 * (s + 1)],
                    ident[0:65, 0:65])
                rc = rc_pool.tile([128, 1], F32, tag="rc",
                                  name=f"rc{hp}{xc}{h2}{s}")
                nc.vector.reciprocal(rc[:], tp[:, 64:65])
                nc.vector.tensor_scalar_mul(
                    out_sb[:, 4 * xc + s, 64 * h:64 * (h + 1)],
                    tp[:, 0:64], rc[:])

        def drain_ship(xc):
            # all heads of x-tiles 4xc..4xc+3 done: one batched out DMA
            nc.sync.dma_start(out_d.ap()[:, 4 * xc:4 * (xc + 1)],
                              out_sb[:, 4 * xc:4 * (xc + 1)])

        # ---- stream 1: (0,0). PE-bound: carries all V tiles + kT ct0
        # chunks 1-7 + qt(0,1). kt chunk c ready just before scores(4c).
        qt_proj(0, 0)
        kt_chunk(0, 0)
        for tt in range(TT):
            scores(0, 0, tt)
            if tt > 0:
                oprime(0, 0, tt - 1)
            v_tile(tt)
            if tt < 27 and tt % 4 == 2:
                kt_chunk(0, 1 + tt // 4)
            if tt == 17:
                qt_proj(0, 1)

        # ---- streams 2-4: exp-paced; previous stream's last O' + drain
        # ride in steps 0-6; kt ct1 / qt fillers resume from step 7 ----
        def stream(hp, xc, prev, fillers, ship_prev_xc=None):
            for tt in range(TT):
                scores(hp, xc, tt)
                if tt == 1:
                    oprime(*prev, TT - 1)
                elif tt == 2:
                    ots = drain_copy(*prev)
                elif tt == 3:
                    oprime(hp, xc, 0)
                    oprime(hp, xc, 1)
                    drain_sub(*prev, ots, 0)
                elif tt == 4:
                    oprime(hp, xc, 2)
                    oprime(hp, xc, 3)
                    drain_sub(*prev, ots, 1)
                elif tt == 5:
                    oprime(hp, xc, 4)
                    drain_sub(*prev, ots, 2)
                elif tt == 6:
                    oprime(hp, xc, 5)
                    drain_sub(*prev, ots, 3)
                    if ship_prev_xc is not None:
                        drain_ship(ship_prev_xc)
                elif tt > 6:
                    oprime(hp, xc, tt - 1)
                    if fillers:
                        fillers.pop(0)()

        f2 = []
        for c in range(4):
            f2 += proj_halves("k", 1, c)
        f2 += proj_halves("q", 1, 0)
        for c in range(4, 7):
            f2 += proj_halves("k", 1, c)
        stream(0, 1, (0, 0), f2)
        f3 = proj_halves("k", 1, 7) + proj_halves("q", 1, 1)
        stream(1, 0, (0, 1), f3)
        stream(1, 1, (1, 0), [], ship_prev_xc=0)

        # ---- final drain (exposed tail) ----
        oprime(1, 1, TT - 1)
        ots = drain_copy(1, 1)
        for s in range(4):
            drain_sub(1, 1, ots, s)
        drain_ship(0)
        drain_ship(1)

    nc.compile()
    return nc


def get_program():
    if "nc" not in _CACHE:
        _CACHE["nc"] = _build_program()
    return _CACHE["nc"]


def _shard_inputs(previous_output, context, Wq, bq, Wk, bk, Wv, bv):
    bf = ml_dtypes.bfloat16
    # x (X, D) -> [p, xc, dt, j] with x[512*xc+j, 128*dt+p]
    xt = [np.ascontiguousarray(
        previous_output[n].reshape(2, 512, KT, 128)
        .transpose(3, 0, 2, 1)).astype(bf) for n in range(N)]
    ctxt = [np.ascontiguousarray(
        context[n].reshape(8, 512, KT, 128)
        .transpose(3, 0, 2, 1)).astype(bf) for n in range(N)]
    in_maps = []
    for c in range(NCORES):
        n, hg = c // CH, c % CH
        sl = slice(CW * hg, CW * (hg + 1))
        # W[sl] (256, 1024) -> [p, dt, col] = W[col, 128*dt+p]
        in_maps.append({
            "xt": xt[n],
            "ctxt": ctxt[n],
            "wqt": np.ascontiguousarray(
                Wq[sl].reshape(CW, KT, 128).transpose(2, 1, 0)).astype(bf),
            "wkt": np.ascontiguousarray(
                Wk[sl].reshape(CW, KT, 128).transpose(2, 1, 0)).astype(bf),
            "wvt": np.ascontiguousarray(
                Wv[sl].reshape(CW, KT, 128).transpose(2, 1, 0)).astype(bf),
            "bq": np.ascontiguousarray(
                bq[sl].reshape(2, 128).T).astype(np.float32),
            "bk": np.ascontiguousarray(
                bk[sl].reshape(2, 128).T).astype(np.float32),
            "bv": np.broadcast_to(
                bv[sl].astype(bf), (128, CW)).copy(),
        })
    return in_maps


LAST_RESULTS = None


def kernel(previous_output, context, Wq, bq, Wk, bk, Wv, bv):
    global LAST_RESULTS
    previous_output = np.asarray(previous_output, dtype=np.float32)
    context = np.asarray(context, dtype=np.float32)
    Wq = np.asarray(Wq, dtype=np.float32)
    Wk = np.asarray(Wk, dtype=np.float32)
    Wv = np.asarray(Wv, dtype=np.float32)
    bq = np.asarray(bq, dtype=np.float32)
    bk = np.asarray(bk, dtype=np.float32)
    bv = np.asarray(bv, dtype=np.float32)

    nc = get_program()
    in_maps = _shard_inputs(previous_output, context, Wq, bq, Wk, bk, Wv, bv)
    res = run_bass_kernel_spmd(nc, in_maps, core_ids=list(range(NCORES)))
    LAST_RESULTS = res

    out = np.empty((N, X, D), dtype=np.float32)
    for c in range(NCORES):
        n, hg = c // CH, c % CH
        # out dram [p, xt, c] -> x = 128*xt + p
        out[n, :, CW * hg:CW * (hg + 1)] = (
            res.results[c]["out"].transpose(1, 0, 2).reshape(X, CW))
    return out
